# revision 29
# baseline (speedup 1.0000x reference)
"""GraphQLayer symmetric fused kernel for 8x trn2 NeuronCores (v2).

Math reduction (as v1): output is rank-1;
  agg[i] = sum_j 1{fl((x_i.x_j)^2) >= 0.85} * s[j],  s[j] = mean_d x[j,d]
  out[i,h] = agg[i] * wsum[h] + b[h]  (diag correction folded in at the end)

Symmetric scheme: each unordered 128-block pair computed ONCE across the
fleet. Core c owns blocks {A : A % 8 == c} stationary (slot-major). Per
[128,512] tile:
  PE   : G = xstat[a]^T @ xt[m]  (fp32, exact threshold semantics)
  ACT  : sq = Square(G)  PSUM->SBUF
  DVE  : msk = (sq >= t) as float32r  (values 0/1, exact; f32r moving dtype
         runs the acc matmul at 1 cyc/row instead of fp32's 4)
  PE   : acc[m] += sca_r[a]^T @ msk   (s_hi/s_lo are bf16-valued, stored
         f32r -> products exact)
  DVE  : orientation-2 stt (sq >= t)*srep accum -> aggA2[:, a]
srep ([128,n] replicated s) is built ON-CHIP from a [1,n] host vector via
K=1 fp32 broadcast matmuls + ACT copies (saves 8MB/core of HBM traffic).
Host combine: sum per-core partials, rank-1 expansion + diag correction.
"""

import sys
from contextlib import ExitStack

import numpy as np

sys.path.insert(0, "/opt/trn_rl_repo")

import concourse.bass as bass  # noqa: E402,F401
import concourse.tile as tile  # noqa: E402
from concourse import bacc, mybir  # noqa: E402
from concourse.bass_utils import run_bass_kernel_spmd  # noqa: E402

D, H = 128, 64
NCORES = 8
THRESH = 0.85

f32 = mybir.dt.float32
f32r = mybir.dt.float32r
bf16 = mybir.dt.bfloat16
fp8 = mybir.dt.float8e4
AOP = mybir.AluOpType
AFT = mybir.ActivationFunctionType
FP8_SCALES = (1.0, 1.0, 256.0, 65536.0)


def _slots_for_chunk(m: int, noct: int):
    """Stationary slots covering moving chunk m (octet o = m//2)."""
    o = m // 2
    slots = [(o + k) % noct for k in range(noct // 2)]
    if o >= noct // 2:
        slots.append((o + noct // 2) % noct)
    return o, slots


def _build_kernel(n: int):
    nc = bacc.Bacc("TRN2", target_bir_lowering=False, debug=False,
                   num_devices=NCORES)
    nslots = n // 1024
    xt_d = nc.dram_tensor("xt", [128, n], f32, kind="ExternalInput").ap()
    srowhl_d = nc.dram_tensor("srowhl", [1, 2 * n], bf16,
                              kind="ExternalInput").ap()
    ones_d = nc.dram_tensor("ones", [1, 128], bf16,
                            kind="ExternalInput").ap()
    xstat_d = nc.dram_tensor("xstat", [128, 128 * nslots], f32,
                             kind="ExternalInput").ap()
    sca8_d = nc.dram_tensor("sca8", [128, 4 * nslots], fp8,
                            kind="ExternalInput").ap()
    out1_d = nc.dram_tensor("out1", [4, n], f32, kind="ExternalOutput").ap()
    out2_d = nc.dram_tensor("out2", [128, nslots], f32,
                            kind="ExternalOutput").ap()

    with tile.TileContext(nc) as tc:
        with ExitStack() as ctx:
            _emit(ctx, tc, n, out1_d, out2_d, xt_d, srowhl_d, ones_d,
                  xstat_d, sca8_d)
    nc.compile()
    return nc


def _emit(ctx, tc, n, out1_d, out2_d, xt_d, srowhl_d, ones_d, xstat_d,
          sca8_d):
    nc = tc.nc
    nchunks = n // 512
    noct = n // 1024
    nslots = noct
    assert nslots % NCORES == 0 or nslots >= 1

    cst = ctx.enter_context(tc.tile_pool(name="cst", bufs=1))
    g_pool = ctx.enter_context(tc.tile_pool(name="gp", bufs=4, space="PSUM"))
    acc_pool = ctx.enter_context(tc.tile_pool(name="accp", bufs=2,
                                              space="PSUM"))
    sq_pool = ctx.enter_context(tc.tile_pool(name="sqp", bufs=6))
    msk_pool = ctx.enter_context(tc.tile_pool(name="mskp", bufs=6))
    scr_pool = ctx.enter_context(tc.tile_pool(name="scrp", bufs=3))
    prt_pool = ctx.enter_context(tc.tile_pool(name="prtp", bufs=3))
    fin_pool = ctx.enter_context(tc.tile_pool(name="finp", bufs=3))

    # --- bulk input loads. SP queue in need-order: srow (feeds the PE
    # broadcast pipeline that fills the initial DMA bubble), then chunk-0
    # slots + xt0, then everything else. scaf rides the ACT queue. ---
    ones_t = cst.tile([1, 128], bf16, tag="ones")
    nc.sync.dma_start(out=ones_t[:], in_=ones_d[:])
    # s hi/lo bf16 rows for the on-chip srep broadcast; tiny DMAs ride the
    # ACT queue so the SP bulk stream is not delayed.
    srb_pool = ctx.enter_context(tc.tile_pool(name="srbp", bufs=4))
    sr_tiles = []
    for j in range(n // 512):
        sr = srb_pool.tile([1, 1024], bf16, tag="sr", name=f"sr{j}")
        nc.scalar.dma_start(out=sr[:],
                            in_=srowhl_d[:, 1024 * j:1024 * (j + 1)])
        sr_tiles.append(sr)
    xstat_t = cst.tile([128, 128 * nslots], f32, tag="xstat")
    nc.sync.dma_start(out=xstat_t[:, 0:128], in_=xstat_d[:, 0:128])
    xt_tiles = []
    for j in range(nchunks):
        t = cst.tile([128, 512], f32, tag=f"xt{j}", name=f"xt{j}")
        xt_tiles.append(t)
    for q in range(4):
        nc.sync.dma_start(
            out=xt_tiles[0][:, 128 * q:128 * q + 128],
            in_=xt_d[:, 128 * q:128 * q + 128])
    for a in range(1, nslots):
        nc.sync.dma_start(out=xstat_t[:, 128 * a:128 * a + 128],
                          in_=xstat_d[:, 128 * a:128 * a + 128])
    for j in range(1, nchunks):
        t = xt_tiles[j]
        if j < 4:
            for q in range(4):
                nc.sync.dma_start(
                    out=t[:, 128 * q:128 * q + 128],
                    in_=xt_d[:, j * 512 + 128 * q:j * 512 + 128 * q + 128])
        else:
            nc.sync.dma_start(out=t[:], in_=xt_d[:, j * 512:(j + 1) * 512])
    sca8_t = cst.tile([128, 4 * nslots], fp8, tag="sca8")
    nc.scalar.dma_start(out=sca8_t[:], in_=sca8_d[:])

    # --- srep built on-chip: 32x K=1 fp32 broadcast matmul + ACT copy ---
    srep_t = cst.tile([128, n], f32, tag="srep")

    def emit_srep(j):
        sr = sr_tiles[j]
        bc = g_pool.tile([128, 512], f32, tag="g", name=f"bc{j}")
        nc.tensor.matmul(out=bc[:], lhsT=ones_t[:], rhs=sr[0:1, 0:512],
                         start=True, stop=False)
        nc.tensor.matmul(out=bc[:], lhsT=ones_t[:], rhs=sr[0:1, 512:1024],
                         start=False, stop=True)
        nc.scalar.copy(srep_t[:, j * 512:(j + 1) * 512], bc[:])

    aggA2 = cst.tile([128, nslots], f32, tag="aggA2")
    nc.vector.memset(aggA2[:], 0.0)

    LAG1 = 2
    q1 = []   # (m, idx, a, msk, acc, nslot, sq)
    drained = set()

    add_i = 0

    def emit_stage1():
        nonlocal add_i
        m, idx, a, msk, acc, nslot, sq = q1.pop(0)
        o = m // 2
        nc.tensor.matmul(out=acc[0:4, :],
                         lhsT=sca8_t[:, 4 * a:4 * a + 4], rhs=msk[:],
                         start=(idx == 0), stop=(idx == nslot - 1))
        if a != o:
            scr = scr_pool.tile([128, 512], f32, tag="scr",
                                name=f"scr_{m}_{a}")
            prt = prt_pool.tile([128, 1], f32, tag="prt",
                                name=f"prt_{m}_{a}")
            nc.vector.scalar_tensor_tensor(
                out=scr[:], in0=sq[:], scalar=THRESH,
                in1=srep_t[:, m * 512:(m + 1) * 512],
                op0=AOP.is_ge, op1=AOP.mult, accum_out=prt[:])
            # aggA2[:, a] += prt; alternate engines to balance load
            if add_i % 2 == 0:
                nc.vector.tensor_tensor(aggA2[:, a:a + 1], prt[:],
                                        aggA2[:, a:a + 1], AOP.add)
            else:
                nc.scalar.activation(aggA2[:, a:a + 1], prt[:, 0:1],
                                     AFT.Identity, bias=aggA2[:, a:a + 1])
            add_i += 1
        if idx == nslot - 1:
            accsm = fin_pool.tile([4, 512], f32, tag="accsm",
                                  name=f"accsm_{m}")
            nc.scalar.copy(accsm[0:4, :], acc[0:4, :])
            nc.sync.dma_start(out=out1_d[0:4, 512 * m:512 * m + 512],
                              in_=accsm[0:4, :])
            drained.add(m)

    for m in range(nchunks):
        emit_srep(m)
        o, slots = _slots_for_chunk(m, noct)
        acc = acc_pool.tile([128, 512], f32, tag="acc", name=f"acc_{m}")
        for idx, a in enumerate(slots):
            g = g_pool.tile([128, 512], f32, tag="g", name=f"g_{m}_{a}")
            nc.tensor.matmul(out=g[:], lhsT=xstat_t[:, 128 * a:128 * a + 128],
                             rhs=xt_tiles[m][:], start=True, stop=True)
            sq = sq_pool.tile([128, 512], f32, tag="sq",
                              name=f"sq_{m}_{a}")
            nc.scalar.activation(sq[:], g[:], AFT.Square)
            msk = msk_pool.tile([128, 512], fp8, tag="msk",
                                name=f"msk_{m}_{a}")
            nc.vector.tensor_scalar(msk[:], sq[:], THRESH, None, AOP.is_ge)
            q1.append((m, idx, a, msk, acc, len(slots), sq))
            if len(q1) > LAG1:
                emit_stage1()
    while q1:
        emit_stage1()
    assert len(drained) == nchunks

    nc.sync.dma_start(out=out2_d[:], in_=aggA2[:])


_CACHE = {}


def _prep(x, n):
    import ml_dtypes
    xt = np.ascontiguousarray(x.T).astype(np.float32)          # [128, n]
    s = (x.astype(np.float64).sum(axis=1) / 128.0).astype(np.float32)
    s_hi = s.astype(ml_dtypes.bfloat16)
    s_lo = (s - s_hi.astype(np.float32)).astype(ml_dtypes.bfloat16)
    # per-chunk packed bf16 [s_hi(512) | s_lo(512)] rows for on-chip srep
    srowhl = np.empty((1, 2 * n), dtype=ml_dtypes.bfloat16)
    hi = s_hi.reshape(-1, 512)
    lo = s_lo.reshape(-1, 512)
    for j in range(n // 512):
        srowhl[0, 1024 * j:1024 * j + 512] = hi[j]
        srowhl[0, 1024 * j + 512:1024 * (j + 1)] = lo[j]
    ones = np.ones((1, 128), dtype=ml_dtypes.bfloat16)

    # 4-term fp8 split of s with per-term scales (recombined on host);
    # residual ~1e-6 relative, and every mask*term product is exact on PE.
    s8 = np.empty((4, n), dtype=ml_dtypes.float8_e4m3fn)
    r = s.astype(np.float64)
    for k, sc in enumerate(FP8_SCALES):
        t8 = (r * sc).astype(ml_dtypes.float8_e4m3fn)
        s8[k] = t8
        r = r - t8.astype(np.float64) / sc

    nslots = n // 1024
    in_maps = []
    for c in range(NCORES):
        xstat = np.empty((128, 128 * nslots), dtype=np.float32)
        sca8 = np.empty((128, 4 * nslots), dtype=ml_dtypes.float8_e4m3fn)
        for a in range(nslots):
            A = 8 * a + c
            xstat[:, 128 * a:128 * a + 128] = xt[:, 128 * A:128 * A + 128]
            for k in range(4):
                sca8[:, 4 * a + k] = s8[k, 128 * A:128 * A + 128]
        in_maps.append({
            "xt": xt,
            "srowhl": srowhl,
            "ones": ones,
            "xstat": xstat,
            "sca8": sca8,
        })
    return in_maps, s


def _combine(results, s, x, W, b, n):
    """Gather/unshard: sum per-core partials, apply rank-1 expansion."""
    nslots = n // 1024
    agg = np.zeros(n, dtype=np.float64)
    for c in range(NCORES):
        p1 = results[c]["out1"].astype(np.float64)        # [4, n]
        agg += (p1[0] / FP8_SCALES[0] + p1[1] / FP8_SCALES[1]
                + p1[2] / FP8_SCALES[2] + p1[3] / FP8_SCALES[3])
        p2 = results[c]["out2"].astype(np.float64)        # [128, nslots]
        idx = (8 * np.arange(nslots)[None, :] + c) * 128 \
            + np.arange(128)[:, None]
        np.add.at(agg, idx.ravel(), p2.ravel())
    # diagonal correction: remove self-edge contributions s_i * 1{fid_ii>=t}
    q = np.einsum("nd,nd->n", x, x, dtype=np.float64).astype(np.float32)
    dmask = np.float32(q * q) >= np.float32(THRESH)
    agg -= np.where(dmask, s, np.float32(0.0))
    wsum = W.astype(np.float64).sum(axis=1).astype(np.float32)
    out = agg.astype(np.float32)[:, None] * wsum[None, :] + b[None, :]
    return out.astype(np.float32)


def kernel(x: np.ndarray, W: np.ndarray, b: np.ndarray,
           trace: bool = False, tmpdir: str | None = None):
    x = np.asarray(x, dtype=np.float32)
    W = np.asarray(W, dtype=np.float32)
    b = np.asarray(b, dtype=np.float32)
    n = x.shape[0]

    in_maps, s = _prep(x, n)

    key = ("nc4", n)
    if key not in _CACHE:
        _CACHE[key] = _build_kernel(n)
    nc = _CACHE[key]

    res = run_bass_kernel_spmd(nc, in_maps, list(range(NCORES)),
                               trace=trace, tmpdir=tmpdir)
    out = _combine(res.results, s, x, W, b, n)
    if trace:
        kernel.last_results = res
    return out
